# revision 1
# baseline (speedup 1.0000x reference)
"""Trainium2 Bass kernel for nn_Contour_to_distance_map.

Math (per polygon p, mesh pixel m=(mx,my), vertex k, with u=c_k-m, v=c_{k+1}-m):
  nd=|u|, nr=|v|, cross = u_y v_x - u_x v_y, dot = u.v
  ang = arccos(clip(dot/(nd nr), -1+eps, 1-eps))
      = pi/2 - 2*arctan(clip(u_half, -U, U)),  u_half = dot/(nd*nr + |cross|)
  (Lagrange: (nd*nr)^2 = cross^2 + dot^2 = X; X is also Q1_k*Q1_{k+1} with
   Q1 = nd^2, whose outer-product expansion is a sum of nonnegatives -> no
   cancellation.)
  winding = |sum_k tanh(1e5*cross)*ang|; out = winding*min_k nd / max(...)

Every per-(pixel,k) field is an outer sum P_k(i) + v_k(j) over row/col
coordinates, so the device evaluates tiny-contraction bf16-3-split matmuls
(exact fp32 reconstruction) plus elementwise passes. Data-parallel over 8
cores: core c -> polygon c//2, row-half c%2. Global-max normalization on host
(ratio is scale-invariant; the 1/2pi cancels).
"""

import numpy as np
import ml_dtypes

import concourse.bass as bass
import concourse.bacc as bacc
import concourse.tile as tile
import concourse.mybir as mybir
import concourse.bass_utils as bass_utils
import concourse.dve_ops as dve_ops
from concourse.dve_ops import AFFINE_MUL_REDUCE, DveOp
from concourse.dve_spec import (Spec, Src0, Src1, C0, C1, Zero, maxx, minn,
                                lower, _has_src1)
from concourse.dve_uop import DveOpSpec
from concourse.tile_rust import add_dep_helper

F32 = mybir.dt.float32
BF16 = mybir.dt.bfloat16
I32 = mybir.dt.int32

SIZE = 256
K = 64
NPAIR = K // 2          # 32 vertex pairs
# PE row-group layout per pair (each block in its own 32-row array group so
# the four matmuls run concurrently on different sub-arrays):
#   rows [ 0:12)  cross (6 bf16-split rows per k)
#   rows [32:44)  dot   (6 per k)
#   rows [64:88)  X     (12 per k: A3 + B3 + {hh,hm,mh} products)
#   rows [96:104) Q1    (4 per k: 2-split)
NROWS = 104
HALF_PAIRS = NPAIR // 2
HCOLS = HALF_PAIRS * 512       # 8192 real elements per half per quantity
EPS = 1e-5
K_SIGN = 100000.0
U_CLIP = float(np.tan(np.arcsin(1.0 - EPS) / 2.0))   # ~0.9955378
MINACC_INIT = 3.0e38

_BF = ml_dtypes.bfloat16


# ---------------- custom fused DVE ops ---------------- #

def _make_op(name, spec):
    """Author + register a custom DVE op at runtime (sha computed here)."""
    for op in dve_ops.OPS:
        if op.name == name:
            return op
    row = dve_ops._CUSTOM_DVE_ROW_BASE + len(dve_ops.OPS)
    assert row < 0x20
    dve_ops._SUB_OPCODE_FOR_NAME[name] = row
    shas = {}
    for ver in ("v3", "v4"):
        try:
            s = DveOpSpec(name=name, opcode=row, uops=lower(spec, ver=ver),
                          rd1_en=_has_src1(spec))
            shas[ver] = s.sha(ver)
        except Exception:
            pass
    op = DveOp(name, spec, subdim=False, uops_sha=shas)
    dve_ops.OPS.append(op)
    dve_ops.CUSTOM_DVE_SPECS[name] = spec
    return op


# g = |in0| + in1
ABS_ADD_ANT = _make_op("ABS_ADD_ANT", Spec(
    body=maxx(Src0, Zero - Src0) + Src1,
    reference=lambda in0, in1, s0, s1, imm2:
        np.abs(in0.astype(np.float32)) + in1,
))

# uc = clip(in0*in1, C1, C0)  (pass s0=+U, s1=-U)
MUL_CLIP_ANT = _make_op("MUL_CLIP_ANT", Spec(
    body=minn(maxx(Src0 * Src1, C1), C0),
    reference=lambda in0, in1, s0, s1, imm2:
        np.minimum(np.maximum(in0.astype(np.float32) * in1, s1), s0),
))


# ---------------- host-side coefficients ---------------- #

def _split3(x):
    """f64 -> three bf16 planes summing to ~fp32 precision."""
    h = np.asarray(x, _BF).astype(np.float64)
    m = np.asarray(x - h, _BF).astype(np.float64)
    l = np.asarray(x - h - m, _BF).astype(np.float64)
    return (h.astype(_BF), m.astype(_BF), l.astype(_BF))


def _core_coeffs(C, core):
    """lhsT (NROWS, NPAIR*128) + rhs (NROWS, NPAIR*2048) bf16 for one core."""
    p, hh = core // 2, core % 2
    mx = (hh * 128 + np.arange(128, dtype=np.float64)) / SIZE
    my = np.arange(SIZE, dtype=np.float64) / SIZE
    cx, cy = C[p, :, 0], C[p, :, 1]
    c1x, c1y = np.roll(cx, -1), np.roll(cy, -1)
    ex, ey = c1x - cx, c1y - cy

    P1 = (cx[None, :] - mx[:, None]) ** 2
    v1 = (cy[None, :] - my[:, None]) ** 2
    P1n = np.roll(P1, -1, axis=1)
    v1n = np.roll(v1, -1, axis=1)
    A = P1 * P1n
    B = v1 * v1n
    P3 = ey[None, :] * mx[:, None] + (cy * ex - cx * ey)[None, :]
    v3 = -ex[None, :] * my[:, None]
    P4 = (cx[None, :] - mx[:, None]) * (c1x[None, :] - mx[:, None])
    v4 = (cy[None, :] - my[:, None]) * (c1y[None, :] - my[:, None])

    sp = {}
    for name, arr in [("P1", P1), ("v1", v1), ("P1n", P1n), ("v1n", v1n),
                      ("A", A), ("B", B), ("P3", P3), ("v3", v3),
                      ("P4", P4), ("v4", v4)]:
        sp[name] = _split3(arr)

    ones_i = np.ones(128, _BF)
    ones_j = np.ones(SIZE, _BF)
    PRODS = [(0, 0), (0, 1), (1, 0)]   # hh, hm, mh split products

    def block_rows(k, blk):
        rows = []
        if blk == 0:    # cross = P3 + v3
            for t in range(3):
                rows.append((sp["P3"][t][:, k], ones_j))
            for t in range(3):
                rows.append((ones_i, sp["v3"][t][:, k]))
        elif blk == 1:  # dot = P4 + v4
            for t in range(3):
                rows.append((sp["P4"][t][:, k], ones_j))
            for t in range(3):
                rows.append((ones_i, sp["v4"][t][:, k]))
        elif blk == 2:  # X = A + B + P1*v1n + P1n*v1 (all nonneg groups)
            for t in range(3):
                rows.append((sp["A"][t][:, k], ones_j))
            for t in range(3):
                rows.append((ones_i, sp["B"][t][:, k]))
            for a, b in PRODS:
                rows.append((sp["P1"][a][:, k], sp["v1n"][b][:, k]))
            for a, b in PRODS:
                rows.append((sp["P1n"][a][:, k], sp["v1"][b][:, k]))
        else:           # Q1 = P1 + v1 (2-split)
            for t in range(2):
                rows.append((sp["P1"][t][:, k], ones_j))
            for t in range(2):
                rows.append((ones_i, sp["v1"][t][:, k]))
        return rows

    BLK_BASE = [0, 32, 64, 96]
    BLK_KROWS = [6, 6, 12, 4]

    lhsT = np.zeros((NROWS, NPAIR, 128), _BF)
    rhs = np.zeros((NROWS, NPAIR, 512), _BF)
    for pp in range(NPAIR):
        for t in range(2):
            k = 2 * pp + t
            for blk in range(4):
                rows = block_rows(k, blk)
                base = BLK_BASE[blk] + t * BLK_KROWS[blk]
                for r, (li, rj) in enumerate(rows):
                    lhsT[base + r, pp, :] = li
                    rhs[base + r, pp, t * 256:(t + 1) * 256] = rj
    return lhsT.reshape(NROWS, -1), rhs.reshape(NROWS, -1)


_PROGRAM = None


def _build_program():
    nc = bacc.Bacc("TRN2", target_bir_lowering=False, debug=False,
                   enable_asserts=False, num_devices=1)
    lhsT_d = nc.dram_tensor("lhsT", [NROWS, NPAIR * 128], BF16,
                            kind="ExternalInput").ap()
    rhs_d = nc.dram_tensor("rhs", [NROWS, NPAIR * 512], BF16,
                           kind="ExternalInput").ap()
    out_d = nc.dram_tensor("pm2", [128, SIZE], F32, kind="ExternalOutput").ap()

    AF = mybir.ActivationFunctionType
    ALU = mybir.AluOpType
    with tile.TileContext(nc, pool_alloc_mode="queue") as tc:
        with tc.tile_pool(name="lhsp", bufs=1) as lhsp, \
             tc.tile_pool(name="rhsp", bufs=3) as rhsp, \
             tc.tile_pool(name="fields", bufs=1) as fieldp, \
             tc.tile_pool(name="fin", bufs=1) as finp, \
             tc.tile_pool(name="ps", bufs=2, space="PSUM") as psp, \
             tc.tile_pool(name="q1ps", bufs=1, space="PSUM") as q1psp:

            lhsT_sb = lhsp.tile([NROWS, NPAIR * 128], BF16)
            # first chunk on the sync queue ahead of the rhs stream (pair 0
            # needs it); the rest in parallel on the gpsimd queue
            NL = 4
            lw = NPAIR * 128 // NL
            nc.sync.dma_start(lhsT_sb[:, 0:lw], lhsT_d[:, 0:lw])
            for c in range(1, NL):
                nc.gpsimd.dma_start(lhsT_sb[:, c * lw:(c + 1) * lw],
                                    lhsT_d[:, c * lw:(c + 1) * lw])

            minacc = finp.tile([128, 1024], F32)
            nc.vector.memset(minacc[:, :], MINACC_INIT)

            wparts = []
            prev_act = None  # last ACT inst of previous set-phase
            for half in range(2):
                # cdf: interleaved [cross(512)|dot(512)] blocks per pair
                cdf = fieldp.tile([128, HALF_PAIRS * 1024], F32, tag="cdf")
                denf = fieldp.tile([128, HCOLS], F32, tag="denf")
                sf = fieldp.tile([128, HCOLS], F32, tag="sf")
                af = fieldp.tile([128, HCOLS], F32, tag="af")

                # ---- streaming: PE matmuls + psum evacuation [sqrt set] ----
                first_act = None
                last_act = None
                q1t = None
                for i in range(HALF_PAIRS):
                    pp = half * HALF_PAIRS + i
                    rhs_t = rhsp.tile([NROWS, 512], BF16, tag="rhs")
                    nc.sync.dma_start(rhs_t[:, :],
                                      rhs_d[:, pp * 512:(pp + 1) * 512])
                    ps = psp.tile([128, 1536], F32, tag="ps")
                    if i % 2 == 0:
                        q1t = q1psp.tile([128, 1024], F32, tag="q1")
                    lt = lhsT_sb[:, pp * 128:(pp + 1) * 128]
                    # four matmuls in distinct PE row-groups -> concurrent
                    nc.tensor.matmul(ps[:, 0:512], lt[0:12, :],
                                     rhs_t[0:12, :], start=True, stop=True)
                    nc.tensor.matmul(ps[:, 512:1024], lt[32:44, :],
                                     rhs_t[32:44, :], start=True, stop=True)
                    nc.tensor.matmul(ps[:, 1024:1536], lt[64:88, :],
                                     rhs_t[64:88, :], start=True, stop=True)
                    nc.tensor.matmul(q1t[:, (i % 2) * 512:(i % 2) * 512 + 512],
                                     lt[96:104, :], rhs_t[96:104, :],
                                     start=True, stop=True,
                                     tile_position=(96, 0))
                    i1 = nc.scalar.activation(cdf[:, i * 1024:(i + 1) * 1024],
                                              ps[:, 0:1024], AF.Copy)
                    i2 = nc.scalar.activation(denf[:, i * 512:(i + 1) * 512],
                                              ps[:, 1024:1536], AF.Sqrt)
                    if i % 2 == 1:
                        nc.vector.tensor_tensor(minacc[:, :], minacc[:, :],
                                                q1t[:, 0:1024], op=ALU.min)
                    if first_act is None:
                        first_act = i1
                    last_act = i2
                if prev_act is not None:
                    add_dep_helper(first_act.ins, prev_act.ins, sync=False,
                                   reason="ACT table-set phase order")
                prev_act = last_act

                # strided views: cross / dot halves of cdf
                def cview(ch, which, width):
                    lo = ch * width
                    v = cdf[:, lo * 2:(ch + 1) * width * 2]
                    v = v.rearrange("p (b q) -> p b q", q=1024)
                    return v[:, :, which * 512:(which + 1) * 512]

                # ---- C phase part 1 [tanh set]: overlaps the DVE-only
                # B phase below (reads only the cross views of cdf) ----
                NCH = 4
                CW = HCOLS // NCH  # 2048
                tanh_insts = []
                for ch in range(NCH):
                    sl = slice(ch * CW, (ch + 1) * CW)
                    it = nc.scalar.activation(sf[:, sl], cview(ch, 0, CW),
                                              AF.Tanh, scale=K_SIGN)
                    tanh_insts.append(it)
                add_dep_helper(tanh_insts[0].ins, prev_act.ins, sync=False,
                               reason="ACT table-set phase order")

                # ---- B phase (DVE only) ----
                for ch in range(NCH):
                    sl = slice(ch * CW, (ch + 1) * CW)
                    # g = |cross| + den   (in place over denf)
                    nc.vector._custom_dve(ABS_ADD_ANT, out=denf[:, sl],
                                          in0=cview(ch, 0, CW),
                                          in1=denf[:, sl])
                    # rg = 1/g
                    nc.vector.reciprocal_approx_fast(out=denf[:, sl],
                                                     in_=denf[:, sl])
                    # uc = clip(dot*rg)  (in place over cdf dot-blocks)
                    nc.vector._custom_dve(MUL_CLIP_ANT, out=cview(ch, 1, CW),
                                          in0=cview(ch, 1, CW),
                                          in1=denf[:, sl],
                                          s0=U_CLIP, s1=-U_CLIP)

                atan_insts = []
                for ch in range(NCH):
                    sl = slice(ch * CW, (ch + 1) * CW)
                    ia = nc.scalar.activation(af[:, sl], cview(ch, 1, CW),
                                              AF.Arctan)
                    atan_insts.append(ia)
                    # prod = (a*-2 + pi/2)*s  (in place over af)
                    nc.vector._custom_dve(AFFINE_MUL_REDUCE, out=af[:, sl],
                                          in0=af[:, sl], in1=sf[:, sl],
                                          s0=-2.0, s1=float(np.pi / 2))
                add_dep_helper(atan_insts[0].ins, tanh_insts[-1].ins,
                               sync=False, reason="tanh set before atan set")
                prev_act = atan_insts[-1]

                # ksum tournament fold -> wp (128, 512). Half 0's folds ride
                # the idle GpSimd (hidden under half 1's streaming).
                eng = nc.gpsimd if half == 0 else nc.vector
                eng.tensor_tensor(af[:, 0:2048], af[:, 0:2048],
                                  af[:, 2048:4096], op=ALU.add)
                eng.tensor_tensor(af[:, 4096:6144], af[:, 4096:6144],
                                  af[:, 6144:8192], op=ALU.add)
                eng.tensor_tensor(af[:, 0:2048], af[:, 0:2048],
                                  af[:, 4096:6144], op=ALU.add)
                eng.tensor_tensor(af[:, 0:1024], af[:, 0:1024],
                                  af[:, 1024:2048], op=ALU.add)
                wp = finp.tile([128, 512], F32, tag=f"wp{half}")
                eng.tensor_tensor(wp[:, :], af[:, 0:512], af[:, 512:1024],
                                  op=ALU.add)
                wparts.append(wp)

            # ---- finals (minq folds first: they only depend on the kmin
            # accumulator, so they overlap the C2 tail) ----
            minq = finp.tile([128, 256], F32)
            nc.vector.tensor_tensor(minq[:, :], minacc[:, 0:256],
                                    minacc[:, 256:512], op=ALU.min)
            nc.vector.tensor_tensor(minq[:, :], minq[:, :],
                                    minacc[:, 512:768], op=ALU.min)
            nc.vector.tensor_tensor(minq[:, :], minq[:, :],
                                    minacc[:, 768:1024], op=ALU.min)
            w = finp.tile([128, 512], F32)
            nc.vector.tensor_tensor(w[:, :], wparts[0][:, :], wparts[1][:, :],
                                    op=ALU.add)
            wsum = finp.tile([128, 256], F32)
            nc.vector.tensor_tensor(wsum[:, :], w[:, 0:256], w[:, 256:512],
                                    op=ALU.add)
            nc.vector.tensor_tensor(wsum[:, :], wsum[:, :], wsum[:, :],
                                    op=ALU.mult)
            nc.vector.tensor_tensor(wsum[:, :], wsum[:, :], minq[:, :],
                                    op=ALU.mult)
            nc.sync.dma_start(out_d[:, :], wsum[:, :])

    nc.compile()
    return nc


def _get_program():
    global _PROGRAM
    if _PROGRAM is None:
        _PROGRAM = _build_program()
    return _PROGRAM


def kernel(contour: np.ndarray) -> np.ndarray:
    contour = np.asarray(contour)
    b, n, k, _ = contour.shape
    assert (b, n, k) == (2, 2, K)
    C = contour.reshape(b * n, K, 2).astype(np.float64)

    nc = _get_program()
    in_maps = []
    for core in range(8):
        lhsT, rhs = _core_coeffs(C, core)
        in_maps.append({"lhsT": lhsT, "rhs": rhs})

    res = bass_utils.run_bass_kernel_spmd(nc, in_maps, core_ids=list(range(8)))

    pm2 = np.stack([res.results[c]["pm2"] for c in range(8)])  # (8,128,256)
    pm = np.sqrt(np.maximum(pm2.astype(np.float64), 0.0))
    dmap = (pm / pm.max()).astype(np.float32)
    out = np.zeros((b * n, SIZE, SIZE), np.float32)
    for core in range(8):
        p, hh = core // 2, core % 2
        out[p, hh * 128:(hh + 1) * 128, :] = dmap[core]
    return out.reshape(b, n, SIZE, SIZE)



# revision 5
# speedup vs baseline: 4.0782x; 4.0782x over previous
"""Trainium2 Bass kernel for nn_Contour_to_distance_map.

Reformulation: the reference's |sum_k tanh(1e5*cross)*arccos(...)|/2pi is the
integer winding number n(pixel), computable exactly by ray casting.  For a ray
along +y at row x=mx_i, edge k contributes dir_k if it straddles mx_i and its
intersection y_int lies above my_j.  All per-(row,edge) quantities (straddle,
dir, y_int bucket) are O(S*K) host work; the device recovers the full map by a
suffix-cumsum over a 256-bucket crossing histogram h[b,i]:
    n(i,j) = sum_{b>=j} h[b,i]  ->  one matmul  n = h^T-contracted with Tri[b,j]=[b>=j].

The distance term min_k |c_k - m| stays O(S^2 K) on device: per-vertex squared
distance Q1_k(i,j) = (cx_k-mx_i)^2 + (cy_k-my_j)^2 is an outer sum, evaluated
as tiny-contraction bf16 matmuls (4 vertices per 1024-col matmul), reduced by
a pairwise DVE min from PSUM and a bf16 min tree.

Output per core: pm2 = n^2 * min_k nd^2; host does sqrt + global max norm
(scale-invariant).  Data-parallel: core c -> polygon c//2, row-half c%2.
"""

import numpy as np
import ml_dtypes

import concourse.bass as bass
import concourse.bacc as bacc
import concourse.tile as tile
import concourse.mybir as mybir
import concourse.bass_utils as bass_utils

F32 = mybir.dt.float32
BF16 = mybir.dt.bfloat16

SIZE = 256
K = 64
NCHUNK = 16            # 4 vertices per chunk
_BF = ml_dtypes.bfloat16

# Tri[b, j] = [b >= j]; split into two 128-row groups, concat on free axis.
_TRI = None


def _tri():
    global _TRI
    if _TRI is None:
        b = np.arange(128)
        j = np.arange(SIZE)
        t0 = (b[:, None] >= j[None, :]).astype(_BF)
        t1 = ((128 + b[:, None]) >= j[None, :]).astype(_BF)
        _TRI = np.concatenate([t0, t1], axis=1)  # (128, 512)
    return _TRI


def _core_coeffs(C, core):
    """Inputs for one core: distance-matmul coeffs + crossing histogram."""
    p, hh = core // 2, core % 2
    mx = (hh * 128 + np.arange(128, dtype=np.float64)) / SIZE
    my = np.arange(SIZE, dtype=np.float64) / SIZE
    cx, cy = C[p, :, 0], C[p, :, 1]
    c1x, c1y = np.roll(cx, -1), np.roll(cy, -1)

    P1 = (cx[None, :] - mx[:, None]) ** 2       # (128, K)
    v1 = (cy[None, :] - my[:, None]) ** 2       # (SIZE, K)
    P1b = P1.astype(_BF)
    v1b = v1.astype(_BF)

    lhsT = np.zeros((8, NCHUNK * 128), _BF)
    rhs = np.zeros((8, NCHUNK * 1024), _BF)
    for c in range(NCHUNK):
        for kk in range(4):
            k = 4 * c + kk
            lhsT[2 * kk, c * 128:(c + 1) * 128] = P1b[:, k]
            lhsT[2 * kk + 1, c * 128:(c + 1) * 128] = 1.0
            base = c * 1024 + kk * 256
            rhs[2 * kk, base:base + 256] = 1.0
            rhs[2 * kk + 1, base:base + 256] = v1b[:, k]

    h = np.zeros((256, 128), np.float64)
    for k in range(K):
        dxk = c1x[k] - cx[k]
        lo, hi = min(cx[k], c1x[k]), max(cx[k], c1x[k])
        idx = np.where((mx >= lo) & (mx < hi))[0]
        if len(idx) == 0:
            continue
        d = 1.0 if dxk > 0 else -1.0
        yint = cy[k] + (mx[idx] - cx[k]) * (c1y[k] - cy[k]) / dxk
        B = np.clip(np.floor(yint * SIZE).astype(int), 0, 255)
        np.add.at(h, (B, idx), d)
    hb = h.astype(_BF)                           # counts <= 64: exact
    hcat = np.concatenate([hb[0:128, :], hb[128:256, :]], axis=1)  # (128, 256)

    return {"lhsT": lhsT, "rhs": rhs, "h": hcat, "tri": _tri()}


_PROGRAM = None


def _build_program():
    nc = bacc.Bacc("TRN2", target_bir_lowering=False, debug=False,
                   enable_asserts=False, num_devices=1)
    lhsT_d = nc.dram_tensor("lhsT", [8, NCHUNK * 128], BF16,
                            kind="ExternalInput").ap()
    rhs_d = nc.dram_tensor("rhs", [8, NCHUNK * 1024], BF16,
                           kind="ExternalInput").ap()
    h_d = nc.dram_tensor("h", [128, 256], BF16, kind="ExternalInput").ap()
    tri_d = nc.dram_tensor("tri", [128, 512], BF16, kind="ExternalInput").ap()
    n_d = nc.dram_tensor("nmap", [128, SIZE], F32, kind="ExternalOutput").ap()
    mq_d = nc.dram_tensor("minq", [128, SIZE], BF16,
                          kind="ExternalOutput").ap()

    ALU = mybir.AluOpType
    AF = mybir.ActivationFunctionType
    MINACC_INIT = 3.0e38
    with tile.TileContext(nc, pool_alloc_mode="queue") as tc:
        with tc.tile_pool(name="const", bufs=1) as constp, \
             tc.tile_pool(name="rhsp", bufs=4) as rhsp, \
             tc.tile_pool(name="ebfp", bufs=3) as ebfp, \
             tc.tile_pool(name="finp", bufs=1) as finp, \
             tc.tile_pool(name="ps", bufs=3, space="PSUM") as psp, \
             tc.tile_pool(name="nps", bufs=1, space="PSUM") as npsp:

            # dummy activation first: its ACT table load (~2.7us) overlaps
            # the input DMAs / first matmuls
            dummy = finp.tile([128, 1], BF16)
            nc.vector.memset(dummy[:, :], 0.0)
            nc.scalar.activation(dummy[:, :], dummy[:, :], AF.Copy)

            lhsT_sb = constp.tile([8, NCHUNK * 128], BF16)
            h_sb = constp.tile([128, 256], BF16)
            tri_sb = constp.tile([128, 512], BF16)
            nc.gpsimd.dma_start(h_sb[:, :], h_d[:, :])
            nc.gpsimd.dma_start(tri_sb[:, :], tri_d[:, :])
            nc.gpsimd.dma_start(lhsT_sb[:, :], lhsT_d[:, :])

            macc_a = finp.tile([128, 512], BF16)
            macc_b = finp.tile([128, 512], BF16)
            nc.vector.memset(macc_a[:, :], MINACC_INIT)
            nc.vector.memset(macc_b[:, :], MINACC_INIT)

            # winding: n[i, j] = sum_b h[b, i] * Tri[b, j]
            nps = npsp.tile([128, 256], F32)
            nc.tensor.matmul(nps[:, :], h_sb[:, 0:128], tri_sb[:, 0:256],
                             start=True, stop=False)
            nc.tensor.matmul(nps[:, :], h_sb[:, 128:256], tri_sb[:, 256:512],
                             start=False, stop=True)

            for c in range(NCHUNK):
                rhs_t = rhsp.tile([8, 1024], BF16, tag="rhs")
                nc.sync.dma_start(rhs_t[:, :],
                                  rhs_d[:, c * 1024:(c + 1) * 1024])
                ps = psp.tile([128, 1024], F32, tag="ps")
                nc.tensor.matmul(ps[:, 0:512],
                                 lhsT_sb[:, c * 128:(c + 1) * 128],
                                 rhs_t[:, 0:512], start=True, stop=True)
                nc.tensor.matmul(ps[:, 512:1024],
                                 lhsT_sb[:, c * 128:(c + 1) * 128],
                                 rhs_t[:, 512:1024], start=True, stop=True)
                # ACT evacuates half the PSUM chunk to bf16; DVE running-mins
                # the bf16 half at 2x and the remaining PSUM half at 1x.
                ebf = ebfp.tile([128, 512], BF16, tag="ebf")
                nc.scalar.activation(ebf[:, :], ps[:, 0:512], AF.Copy)
                nc.vector.tensor_tensor(macc_a[:, :], macc_a[:, :],
                                        ebf[:, :], op=ALU.min)
                nc.vector.tensor_tensor(macc_b[:, :], macc_b[:, :],
                                        ps[:, 512:1024], op=ALU.min)

            nc.vector.tensor_tensor(macc_a[:, :], macc_a[:, :],
                                    macc_b[:, :], op=ALU.min)
            nc.vector.tensor_tensor(macc_a[:, 0:256], macc_a[:, 0:256],
                                    macc_a[:, 256:512], op=ALU.min)
            nc.sync.dma_start(mq_d[:, :], macc_a[:, 0:256])

            n_sb = finp.tile([128, 256], F32)
            nc.vector.tensor_copy(n_sb[:, :], nps[:, :])
            nc.sync.dma_start(n_d[:, :], n_sb[:, :])

    nc.compile()
    return nc


def _get_program():
    global _PROGRAM
    if _PROGRAM is None:
        _PROGRAM = _build_program()
    return _PROGRAM


def kernel(contour: np.ndarray) -> np.ndarray:
    contour = np.asarray(contour)
    b, n, k, _ = contour.shape
    assert (b, n, k) == (2, 2, K)
    C = contour.reshape(b * n, K, 2).astype(np.float64)

    nc = _get_program()
    in_maps = [_core_coeffs(C, core) for core in range(8)]
    res = bass_utils.run_bass_kernel_spmd(nc, in_maps, core_ids=list(range(8)))

    nmap = np.stack([res.results[c]["nmap"] for c in range(8)])  # (8,128,256)
    minq = np.stack([res.results[c]["minq"] for c in range(8)])
    pm = np.abs(nmap.astype(np.float64)) * \
        np.sqrt(np.maximum(minq.astype(np.float64), 0.0))
    dmap = (pm / pm.max()).astype(np.float32)
    out = np.zeros((b * n, SIZE, SIZE), np.float32)
    for core in range(8):
        p, hh = core // 2, core % 2
        out[p, hh * 128:(hh + 1) * 128, :] = dmap[core]
    return out.reshape(b, n, SIZE, SIZE)
